# revision 17
# baseline (speedup 1.0000x reference)
"""Trainium2 Bass kernel for nn_ConvShiftLayer, v3.

Per batch element n (1 per NeuronCore, 8 cores):
    h = x[n] @ W_dense                                 (2048, 2048)
    y[t, o] = sum_{d=0..7} h[t-d, (o+d) % 2048]        (h[<0] = 0)
    a = tanh(y),  z = (y > 0)

v3 changes vs v2:
  - conv factorized log2-style: with (A_d f)[t,o] = f[t-d, o+d],
        y = (A0+A1)(A0+A2)(A0+A4) h
    Each stage is ONE PE shift-matmul (T_d row shift, +d col offset on
    the rhs) plus ONE row-aligned DVE add folding the identity term:
    3 shift MMs/chunk instead of 5 -> 44 MMs/tile vs 52.
  - device emits ONLY y (bf16). a = tanh(y) and z = (y > 0) are
    computed on the host from bf16 y (bf16 rounding is monotonic and
    sign-preserving, so z matches the device-fp32 z exactly).
  - lag-3 software pipeline: step s runs dense(s) | sa(s-1) | sb(s-2)
    | y(s-3), one (shift-MM, add) trio interleaved after each dense
    chunk, so no shift MM ever waits on a same-step DVE add.
  - prologue interleaves tiles 0-2's dense per W column chunk so the
    PE stays busy during the 8 MB W load.
  - PSUM: dense per-chunk 2 banks + 6-bank shift pool = 8.
  - output DMA on the gpsimd queue (inputs on the sync queue).
"""

import sys

if "/opt/trn_rl_repo" not in sys.path:
    sys.path.insert(0, "/opt/trn_rl_repo")

import numpy as np

B, L, DIN, F = 8, 2048, 1024, 2048
WC = 8            # conv taps
PAD = WC - 1      # 7
TS = 128 - PAD    # 121 output rows per time tile
NT = (L + TS - 1) // TS   # 17 time tiles
NCH = 4           # channel chunks of 512
CW = 512
NCORES = 8
KD = DIN // 128   # 8 K-tiles

SHIFTS = (4, 2, 1)             # stage shift amounts (T_4, T_2, T_1)
HSW = F + 7       # h tile cols: 2048 + 7 wrap (chain reads up to +7)
SASW = F + 3      # sa tile cols: 2048 + 3 wrap (sb reads +2, y +1)
SBSW = F + 1      # sb tile cols: 2048 + 1 wrap (y reads +1)

_CACHE = {}


def _build_consts():
    # cst[128, 384]: T_4 at [0:128), T_2 at [128:256), T_1 at [256:384)
    # T_d as lhsT: out[m] = rhs[m-d] (zero rows m<d give the h[t<0]=0
    # edge behavior for tile 0).
    c = np.zeros((128, 384), np.float32)
    for j, d in enumerate(SHIFTS):
        for m in range(128):
            if m - d >= 0:
                c[m - d, j * 128 + m] = 1.0
    return c


def _split_matmul_waits(nc):
    """This walrus build accepts only one sync-wait command per instruction;
    hoist extra waits onto preceding same-engine no-ops (one wait each)."""
    import concourse.mybir as mybir

    for fn in nc.m.functions:
        for blk in fn.blocks:
            newl = []
            for inst in blk.instructions:
                si = getattr(inst, "sync_info", None)
                if (
                    si is not None
                    and len(si.on_wait) > 1
                    and not isinstance(inst, mybir.InstNoOp)
                    and getattr(inst, "engine", None) is not None
                ):
                    waits = list(si.on_wait)
                    for wi, w in enumerate(waits[:-1]):
                        pre = mybir.InstNoOp(
                            name=f"{inst.name}_wsplit{wi}",
                            sync_info=mybir.SyncInfo(on_wait=[w], on_update=[]),
                            bass_nofuse=True,
                            engine=inst.engine,
                        )
                        newl.append(pre)
                    si.on_wait = waits[-1:]
                newl.append(inst)
            blk.instructions = newl


def _tile_geom(i):
    # uniform tiles: last tile overlaps tile 15 (identical values re-written)
    # so every tile outputs My=121 rows — the narrow-dtype consumer/DMA
    # path miscomputes on the hardware for shorter tiles.
    t0 = min(TS * i, L - TS)
    My = TS
    hlo = 0 if i == 0 else t0 - PAD
    Mh = min(L, t0 + TS) - hlo
    return t0, My, hlo, Mh


def _build_nc():
    import concourse.bass as bass
    import concourse.mybir as mybir
    from concourse import tile

    f32 = mybir.dt.float32
    bf16 = mybir.dt.bfloat16
    mmdt = mybir.dt.float32r

    nc = bass.Bass("TRN2", target_bir_lowering=False, debug=False)

    # pre-blocked x windows: row block i holds [128 p, 8 k x 128 c] with
    # element (p, 128k+c) = x[hlo_i + c, 128k + p]
    xt_d = nc.declare_dram_parameter("xtb", [NT * 128, DIN], f32, isOutput=False)
    w_d = nc.declare_dram_parameter("w", [DIN, F], f32, isOutput=False)
    cst_d = nc.declare_dram_parameter("cst", [128, 384], f32, isOutput=False)
    y_d = nc.declare_dram_parameter("y", [L, F], bf16, isOutput=True)

    with tile.TileContext(nc) as tc:
        with (
            tc.tile_pool(name="wpool", bufs=1) as wpool,
            tc.tile_pool(name="cpool", bufs=1) as cpool,
            tc.tile_pool(name="xtp", bufs=6) as xtp,
            tc.tile_pool(name="hpool", bufs=4) as hpool,
            tc.tile_pool(name="saspool", bufs=3) as saspool,
            tc.tile_pool(name="sbspool", bufs=3) as sbspool,
            tc.tile_pool(name="ybpool", bufs=3) as ybpool,
            tc.tile_pool(name="hppool", bufs=2, space="PSUM") as hppool,
            tc.tile_pool(name="shpool", bufs=6, space="PSUM") as shpool,
        ):
            halfd = DIN // 2
            xts = [None] * NT

            def dma_xts(i):
                xts[i] = xtp.tile([128, DIN], mmdt, tag="xts", name=f"xts{i}")
                nc.sync.dma_start(
                    xts[i][:, :],
                    xt_d[i * 128 : (i + 1) * 128, :].bitcast(mmdt),
                )

            wt = []
            for k in range(KD):
                wt.append(wpool.tile([128, F], mmdt, tag=f"w{k}", name=f"w{k}"))

            def dma_whalf(h):
                # one 0.5 MB descriptor per k-slice; half h covers dense
                # column chunks 2h and 2h+1
                c0 = h * 2 * CW
                for k in range(KD):
                    nc.sync.dma_start(
                        wt[k][:, c0 : c0 + 2 * CW],
                        w_d[k * 128 : (k + 1) * 128, c0 : c0 + 2 * CW].bitcast(
                            mmdt
                        ),
                    )

            # input DMA order = first-consumption order; xts0's first
            # k-slice goes alone so the very first LDWEIGHTS can start
            # after only 64 KB + 256 KB of input
            xts[0] = xtp.tile([128, DIN], mmdt, tag="xts", name="xts0")
            nc.sync.dma_start(
                xts[0][:, 0:128], xt_d[0:128, 0:128].bitcast(mmdt)
            )
            nc.sync.dma_start(
                wt[0][:, 0:CW], w_d[0:128, 0:CW].bitcast(mmdt)
            )
            nc.sync.dma_start(
                xts[0][:, 128:DIN], xt_d[0:128, 128:DIN].bitcast(mmdt)
            )
            nc.sync.dma_start(
                wt[0][:, CW : 2 * CW], w_d[0:128, CW : 2 * CW].bitcast(mmdt)
            )
            for k in range(1, KD):
                nc.sync.dma_start(
                    wt[k][:, 0 : 2 * CW],
                    w_d[k * 128 : (k + 1) * 128, 0 : 2 * CW].bitcast(mmdt),
                )
            dma_xts(1)
            cst = cpool.tile([128, 384], mmdt, tag="cst")
            nc.sync.dma_start(cst[:], cst_d[:].bitcast(mmdt))
            dma_xts(2)
            dma_xts(3)
            dma_whalf(1)  # W half 0 already loaded piecewise above
            dma_xts(4)
            dma_xts(5)

            hs = [None] * NT
            sas = [None] * NT
            sbs = [None] * NT
            ybf = [None] * NT

            def lhsT(stage, Mh):
                # stage's shift matrix T_{SHIFTS[stage]} as lhsT [Mh, Mh]
                return cst[0:Mh, stage * 128 : stage * 128 + Mh]

            def dense_chunk(i, n):
                # dense(i, n): 8 accumulating k-MMs into one PSUM bank,
                # then scalar-drain to the h SBUF tile (+ wrap after n=3).
                _, _, _, Mh = _tile_geom(i)
                if hs[i] is None:
                    hs[i] = hpool.tile([128, HSW], mmdt, tag="hs", name=f"hs{i}")
                hp = hppool.tile([128, CW], f32, tag="hp")
                for k in range(KD):
                    nc.tensor.matmul(
                        hp[0:Mh, :],
                        xts[i][:, k * 128 : k * 128 + Mh],
                        wt[k][:, n * CW : (n + 1) * CW],
                        start=(k == 0),
                        stop=(k == KD - 1),
                    )
                nc.scalar.copy(hs[i][0:Mh, n * CW : (n + 1) * CW], hp[0:Mh, :])
                if n == NCH - 1:
                    nc.scalar.copy(hs[i][0:Mh, F:HSW], hs[i][0:Mh, 0 : HSW - F])

            def shift(i, stage, n):
                # stage 0: sa = h + A4 h; 1: sb = sa + A2 sa; 2: y = sb + A1 sb
                # one PE shift-MM into PSUM + one row-aligned DVE add.
                t0, My, hlo, Mh = _tile_geom(i)
                d = SHIFTS[stage]
                src = (hs, sas, sbs)[stage][i]
                sp = shpool.tile([128, CW], f32, tag="sp")
                nc.tensor.matmul(
                    sp[0:Mh, :],
                    lhsT(stage, Mh),
                    src[0:Mh, n * CW + d : n * CW + d + CW],
                    start=True,
                    stop=True,
                )
                if stage < 2:
                    dstl, wid, pool, tg = (
                        (sas, SASW, saspool, "sas")
                        if stage == 0
                        else (sbs, SBSW, sbspool, "sbs")
                    )
                    if dstl[i] is None:
                        dstl[i] = pool.tile(
                            [128, wid], mmdt, tag=tg, name=f"{tg}{i}"
                        )
                    dst = dstl[i]
                    nc.vector.tensor_tensor(
                        dst[0:Mh, n * CW : (n + 1) * CW],
                        src[0:Mh, n * CW : (n + 1) * CW].bitcast(f32),
                        sp[0:Mh, :],
                        mybir.AluOpType.add,
                    )
                    if n == NCH - 1:
                        nc.scalar.copy(dst[0:Mh, F:wid], dst[0:Mh, 0 : wid - F])
                else:
                    if ybf[i] is None:
                        ybf[i] = ybpool.tile([128, F], bf16, tag="ybf", name=f"yb{i}")
                    yb = ybf[i]
                    nc.vector.tensor_tensor(
                        yb[0:Mh, n * CW : (n + 1) * CW],
                        src[0:Mh, n * CW : (n + 1) * CW].bitcast(f32),
                        sp[0:Mh, :],
                        mybir.AluOpType.add,
                    )
                    # ship each half as soon as its adds are done so the
                    # last tile's DMA tail is short
                    if n == 1 or n == NCH - 1:
                        mlo = Mh - TS  # 0 for tile 0, 7 otherwise
                        cl = slice((n - 1) * CW, (n + 1) * CW)
                        nc.gpsimd.dma_start(
                            y_d[t0 : t0 + TS, cl], yb[mlo : mlo + TS, cl]
                        )

            # --- prologue: tiles 0-3 dense column-major (32 MMs per W
            # column oversubscribe the W-load DMA window); shifts
            # sa(0..2), sb(0..1), y(0) trail one column behind ---
            for n in range(NCH):
                for i in range(4):
                    dense_chunk(i, n)
                if n >= 1:
                    for i in range(3):
                        shift(i, 0, n - 1)
                if n >= 2:
                    for i in range(2):
                        shift(i, 1, n - 2)
                if n >= 3:
                    shift(0, 2, n - 3)
            for i in range(3):
                shift(i, 0, 3)
            for i in range(2):
                shift(i, 1, 2)
            shift(0, 2, 1)
            for i in range(2):
                shift(i, 1, 3)
            shift(0, 2, 2)
            shift(0, 2, 3)

            # --- steady: step s = dense(s) | sa(s-1) | sb(s-2) | y(s-3);
            # every shift MM depends only on previous-step DVE output.
            # The last three steps each pull one stage-quad forward
            # (sa same-step after the h copies, which is scalar- not
            # DVE-coupled) so the end-of-pipeline backlog halves. ---
            for s in range(4, NT - 3):
                if s + 2 < NT:
                    dma_xts(s + 2)
                for n in range(NCH):
                    dense_chunk(s, n)
                    shift(s - 1, 0, n)
                    shift(s - 2, 1, n)
                    shift(s - 3, 2, n)

            s = NT - 3      # + sa(s) same-step
            dma_xts(s + 2)
            for n in range(NCH):
                dense_chunk(s, n)
                shift(s - 1, 0, n)
                shift(s - 2, 1, n)
                shift(s - 3, 2, n)
                if n >= 2:
                    shift(s, 0, n - 2)
            shift(s, 0, 2)
            shift(s, 0, 3)

            s = NT - 2      # sa(s) same-step + sb(s-1) extra
            for n in range(NCH):
                dense_chunk(s, n)
                shift(s - 2, 1, n)
                shift(s - 3, 2, n)
                shift(s - 1, 1, n)
                if n >= 2:
                    shift(s, 0, n - 2)
            shift(s, 0, 2)
            shift(s, 0, 3)

            s = NT - 1      # sa(s) same-step + y(s-2) extra
            for n in range(NCH):
                dense_chunk(s, n)
                shift(s - 1, 1, n)
                shift(s - 3, 2, n)
                shift(s - 2, 2, n)
                if n >= 2:
                    shift(s, 0, n - 2)
            shift(s, 0, 2)
            shift(s, 0, 3)

            # --- drain: sb(16) | y(15), then y(16) ---
            for n in range(NCH):
                shift(NT - 1, 1, n)
                shift(NT - 2, 2, n)
            for n in range(NCH):
                shift(NT - 1, 2, n)

    _split_matmul_waits(nc)
    return nc


def _get_nc():
    if "nc" not in _CACHE:
        _CACHE["nc"] = _build_nc()
    return _CACHE["nc"]


def _block_xt(xn):
    # [NT*128, DIN]: block i row p, col 128k+c = x[hlo_i + c, 128k + p]
    xT3 = np.ascontiguousarray(xn.T).reshape(KD, 128, L)  # [k, p, t]
    out = np.empty((NT, 128, DIN), np.float32)
    for i in range(NT):
        _, _, hlo, _ = _tile_geom(i)
        # [k, p, c] -> [p, k, c]
        out[i] = xT3[:, :, hlo : hlo + 128].transpose(1, 0, 2).reshape(128, DIN)
    return out.reshape(NT * 128, DIN)


def _make_in_maps(x, W, b):
    x = np.asarray(x, np.float32)
    W = np.ascontiguousarray(np.asarray(W, np.float32))
    cst = _build_consts()
    return [
        {
            "xtb": _block_xt(x[n]),
            "w": W,
            "cst": cst,
        }
        for n in range(NCORES)
    ]


def kernel(x, W_dense, b_dense):
    from concourse.bass_utils import run_bass_kernel_spmd

    b = np.asarray(b_dense, np.float32)
    nc = _get_nc()
    in_maps = _make_in_maps(x, W_dense, b)
    res = run_bass_kernel_spmd(nc, in_maps, list(range(NCORES))).results

    y = np.stack([np.asarray(res[n]["y"]).astype(np.float32) for n in range(NCORES)])

    if np.any(b):
        # device kernel computes bias-free y; fold bias in on the host
        bext = np.concatenate([b, b[:PAD]])
        win = np.lib.stride_tricks.sliding_window_view(bext, WC).sum(axis=1)  # (F,)
        bias_y = np.broadcast_to(win, (L, F)).copy()
        for t in range(PAD):  # edge rows: only taps d <= t contribute
            bias_y[t] = np.array(
                [bext[o : o + t + 1].sum() for o in range(F)], np.float32
            )
        y = y + bias_y[None]

    a = np.tanh(y)
    z = (y > 0).astype(np.float32)
    return y, a, z


# revision 22
# speedup vs baseline: 1.0167x; 1.0167x over previous
"""Trainium2 Bass kernel for nn_ConvShiftLayer, v3.

Per batch element n (1 per NeuronCore, 8 cores):
    h = x[n] @ W_dense                                 (2048, 2048)
    y[t, o] = sum_{d=0..7} h[t-d, (o+d) % 2048]        (h[<0] = 0)
    a = tanh(y),  z = (y > 0)

v3 changes vs v2:
  - conv factorized log2-style: with (A_d f)[t,o] = f[t-d, o+d],
        y = (A0+A1)(A0+A2)(A0+A4) h
    Each stage is ONE PE shift-matmul (T_d row shift, +d col offset on
    the rhs) plus ONE row-aligned DVE add folding the identity term:
    3 shift MMs/chunk instead of 5 -> 44 MMs/tile vs 52.
  - device emits ONLY y (bf16). a = tanh(y) and z = (y > 0) are
    computed on the host from bf16 y (bf16 rounding is monotonic and
    sign-preserving, so z matches the device-fp32 z exactly).
  - lag-3 software pipeline: step s runs dense(s) | sa(s-1) | sb(s-2)
    | y(s-3), one (shift-MM, add) trio interleaved after each dense
    chunk, so no shift MM ever waits on a same-step DVE add.
  - prologue interleaves tiles 0-2's dense per W column chunk so the
    PE stays busy during the 8 MB W load.
  - PSUM: dense per-chunk 2 banks + 6-bank shift pool = 8.
  - output DMA on the gpsimd queue (inputs on the sync queue).
"""

import sys

if "/opt/trn_rl_repo" not in sys.path:
    sys.path.insert(0, "/opt/trn_rl_repo")

import numpy as np

B, L, DIN, F = 8, 2048, 1024, 2048
WC = 8            # conv taps
PAD = WC - 1      # 7
TS = 128 - PAD    # 121 output rows per time tile
NT = (L + TS - 1) // TS   # 17 time tiles
NCH = 4           # channel chunks of 512
CW = 512
NCORES = 8
KD = DIN // 128   # 8 K-tiles

SHIFTS = (4, 2, 1)             # stage shift amounts (T_4, T_2, T_1)
HSW = F + 7       # h tile cols: 2048 + 7 wrap (chain reads up to +7)
SASW = F + 3      # sa tile cols: 2048 + 3 wrap (sb reads +2, y +1)
SBSW = F + 1      # sb tile cols: 2048 + 1 wrap (y reads +1)

_CACHE = {}


def _build_consts():
    # cst[128, 384]: T_4 at [0:128), T_2 at [128:256), T_1 at [256:384)
    # T_d as lhsT: out[m] = rhs[m-d] (zero rows m<d give the h[t<0]=0
    # edge behavior for tile 0).
    c = np.zeros((128, 384), np.float32)
    for j, d in enumerate(SHIFTS):
        for m in range(128):
            if m - d >= 0:
                c[m - d, j * 128 + m] = 1.0
    return c


def _split_matmul_waits(nc):
    """This walrus build accepts only one sync-wait command per instruction;
    hoist extra waits onto preceding same-engine no-ops (one wait each)."""
    import concourse.mybir as mybir

    for fn in nc.m.functions:
        for blk in fn.blocks:
            newl = []
            for inst in blk.instructions:
                si = getattr(inst, "sync_info", None)
                if (
                    si is not None
                    and len(si.on_wait) > 1
                    and not isinstance(inst, mybir.InstNoOp)
                    and getattr(inst, "engine", None) is not None
                ):
                    waits = list(si.on_wait)
                    for wi, w in enumerate(waits[:-1]):
                        pre = mybir.InstNoOp(
                            name=f"{inst.name}_wsplit{wi}",
                            sync_info=mybir.SyncInfo(on_wait=[w], on_update=[]),
                            bass_nofuse=True,
                            engine=inst.engine,
                        )
                        newl.append(pre)
                    si.on_wait = waits[-1:]
                newl.append(inst)
            blk.instructions = newl


def _tile_geom(i):
    # uniform tiles: last tile overlaps tile 15 (identical values re-written)
    # so every tile outputs My=121 rows — the narrow-dtype consumer/DMA
    # path miscomputes on the hardware for shorter tiles.
    t0 = min(TS * i, L - TS)
    My = TS
    hlo = 0 if i == 0 else t0 - PAD
    Mh = min(L, t0 + TS) - hlo
    return t0, My, hlo, Mh


def _build_nc():
    import concourse.bass as bass
    import concourse.mybir as mybir
    from concourse import tile

    f32 = mybir.dt.float32
    bf16 = mybir.dt.bfloat16
    mmdt = mybir.dt.float32r

    nc = bass.Bass("TRN2", target_bir_lowering=False, debug=False)

    # pre-blocked x windows: row block i holds [128 p, 8 k x 128 c] with
    # element (p, 128k+c) = x[hlo_i + c, 128k + p]
    xt_d = nc.declare_dram_parameter("xtb", [NT * 128, DIN], f32, isOutput=False)
    w_d = nc.declare_dram_parameter("w", [DIN, F], f32, isOutput=False)
    cst_d = nc.declare_dram_parameter("cst", [128, 384], f32, isOutput=False)
    y_d = nc.declare_dram_parameter("y", [L, F], bf16, isOutput=True)

    with tile.TileContext(nc) as tc:
        with (
            tc.tile_pool(name="wpool", bufs=1) as wpool,
            tc.tile_pool(name="cpool", bufs=1) as cpool,
            tc.tile_pool(name="xtp", bufs=6) as xtp,
            tc.tile_pool(name="hpool", bufs=4) as hpool,
            tc.tile_pool(name="saspool", bufs=3) as saspool,
            tc.tile_pool(name="sbspool", bufs=3) as sbspool,
            tc.tile_pool(name="ybpool", bufs=3) as ybpool,
            tc.tile_pool(name="hppool", bufs=4, space="PSUM") as hppool,
            tc.tile_pool(name="shpool", bufs=4, space="PSUM") as shpool,
        ):
            halfd = DIN // 2
            xts = [None] * NT

            def dma_xts(i):
                xts[i] = xtp.tile([128, DIN], mmdt, tag="xts", name=f"xts{i}")
                nc.sync.dma_start(
                    xts[i][:, :],
                    xt_d[i * 128 : (i + 1) * 128, :].bitcast(mmdt),
                )

            wt = []
            for k in range(KD):
                wt.append(wpool.tile([128, F], mmdt, tag=f"w{k}", name=f"w{k}"))

            def dma_wcol(n, k0=0):
                for k in range(k0, KD):
                    nc.sync.dma_start(
                        wt[k][:, n * CW : (n + 1) * CW],
                        w_d[k * 128 : (k + 1) * 128, n * CW : (n + 1) * CW].bitcast(
                            mmdt
                        ),
                    )

            # input DMA order = first-consumption order; xts0's first
            # k-slice goes alone so the very first LDWEIGHTS can start
            # after only 64 KB + 256 KB of input
            xts[0] = xtp.tile([128, DIN], mmdt, tag="xts", name="xts0")
            nc.sync.dma_start(
                xts[0][:, 0:128], xt_d[0:128, 0:128].bitcast(mmdt)
            )
            nc.sync.dma_start(
                wt[0][:, 0:CW], w_d[0:128, 0:CW].bitcast(mmdt)
            )
            nc.sync.dma_start(
                xts[0][:, 128:DIN], xt_d[0:128, 128:DIN].bitcast(mmdt)
            )
            dma_xts(1)
            dma_xts(2)
            dma_xts(3)
            dma_wcol(0, k0=1)
            cst = cpool.tile([128, 384], mmdt, tag="cst")
            nc.sync.dma_start(cst[:], cst_d[:].bitcast(mmdt))
            dma_wcol(1)
            dma_wcol(2)
            dma_wcol(3)
            dma_xts(4)
            dma_xts(5)

            hs = [None] * NT
            sas = [None] * NT
            sbs = [None] * NT
            ybf = [None] * NT

            def lhsT(stage, Mh):
                # stage's shift matrix T_{SHIFTS[stage]} as lhsT [Mh, Mh]
                return cst[0:Mh, stage * 128 : stage * 128 + Mh]

            def dense_chunk(i, n):
                # dense(i, n): 8 accumulating k-MMs into one PSUM bank,
                # then scalar-drain to the h SBUF tile (+ wrap after n=3).
                _, _, _, Mh = _tile_geom(i)
                if hs[i] is None:
                    hs[i] = hpool.tile([128, HSW], mmdt, tag="hs", name=f"hs{i}")
                hp = hppool.tile([128, CW], f32, tag="hp")
                for k in range(KD):
                    nc.tensor.matmul(
                        hp[0:Mh, :],
                        xts[i][:, k * 128 : k * 128 + Mh],
                        wt[k][:, n * CW : (n + 1) * CW],
                        start=(k == 0),
                        stop=(k == KD - 1),
                    )
                nc.scalar.copy(hs[i][0:Mh, n * CW : (n + 1) * CW], hp[0:Mh, :])
                if n == NCH - 1:
                    nc.scalar.copy(hs[i][0:Mh, F:HSW], hs[i][0:Mh, 0 : HSW - F])

            def shift(i, stage, n):
                # stage 0: sa = h + A4 h; 1: sb = sa + A2 sa; 2: y = sb + A1 sb
                # one PE shift-MM into PSUM + one row-aligned DVE add.
                t0, My, hlo, Mh = _tile_geom(i)
                d = SHIFTS[stage]
                src = (hs, sas, sbs)[stage][i]
                sp = shpool.tile([128, CW], f32, tag="sp")
                nc.tensor.matmul(
                    sp[0:Mh, :],
                    lhsT(stage, Mh),
                    src[0:Mh, n * CW + d : n * CW + d + CW],
                    start=True,
                    stop=True,
                )
                if stage < 2:
                    dstl, wid, pool, tg = (
                        (sas, SASW, saspool, "sas")
                        if stage == 0
                        else (sbs, SBSW, sbspool, "sbs")
                    )
                    if dstl[i] is None:
                        dstl[i] = pool.tile(
                            [128, wid], mmdt, tag=tg, name=f"{tg}{i}"
                        )
                    dst = dstl[i]
                    nc.vector.tensor_tensor(
                        dst[0:Mh, n * CW : (n + 1) * CW],
                        src[0:Mh, n * CW : (n + 1) * CW].bitcast(f32),
                        sp[0:Mh, :],
                        mybir.AluOpType.add,
                    )
                    if n == NCH - 1:
                        nc.scalar.copy(dst[0:Mh, F:wid], dst[0:Mh, 0 : wid - F])
                else:
                    if ybf[i] is None:
                        ybf[i] = ybpool.tile([128, F], bf16, tag="ybf", name=f"yb{i}")
                    yb = ybf[i]
                    nc.vector.tensor_tensor(
                        yb[0:Mh, n * CW : (n + 1) * CW],
                        src[0:Mh, n * CW : (n + 1) * CW].bitcast(f32),
                        sp[0:Mh, :],
                        mybir.AluOpType.add,
                    )
                    # ship each half as soon as its adds are done so the
                    # last tile's DMA tail is short
                    if n == 1 or n == NCH - 1:
                        mlo = Mh - TS  # 0 for tile 0, 7 otherwise
                        cl = slice((n - 1) * CW, (n + 1) * CW)
                        nc.gpsimd.dma_start(
                            y_d[t0 : t0 + TS, cl], yb[mlo : mlo + TS, cl]
                        )

            # --- prologue: tiles 0-3 dense, k-major across tiles so each
            # arriving 256 KB W piece feeds 4 MMs (the W-load wire rate
            # is ~1 piece per 0.73 us vs 0.92 us of MMs); shifts
            # sa(0..2), sb(0..1), y(0) trail one column behind ---
            def dense_col_kmajor(tiles, n):
                hps = {}
                for i in tiles:
                    if hs[i] is None:
                        hs[i] = hpool.tile(
                            [128, HSW], mmdt, tag="hs", name=f"hs{i}"
                        )
                    hps[i] = hppool.tile(
                        [128, CW], f32, tag="hp", name=f"hp_p{i}_{n}"
                    )
                for k in range(KD):
                    for i in tiles:
                        _, _, _, Mh = _tile_geom(i)
                        nc.tensor.matmul(
                            hps[i][0:Mh, :],
                            xts[i][:, k * 128 : k * 128 + Mh],
                            wt[k][:, n * CW : (n + 1) * CW],
                            start=(k == 0),
                            stop=(k == KD - 1),
                        )
                for i in tiles:
                    _, _, _, Mh = _tile_geom(i)
                    nc.scalar.copy(
                        hs[i][0:Mh, n * CW : (n + 1) * CW], hps[i][0:Mh, :]
                    )
                    if n == NCH - 1:
                        nc.scalar.copy(
                            hs[i][0:Mh, F:HSW], hs[i][0:Mh, 0 : HSW - F]
                        )

            for n in range(NCH):
                dense_col_kmajor(range(4), n)
                if n >= 1:
                    for i in range(3):
                        shift(i, 0, n - 1)
                if n >= 2:
                    for i in range(2):
                        shift(i, 1, n - 2)
                if n >= 3:
                    shift(0, 2, n - 3)
            for i in range(3):
                shift(i, 0, 3)
            for i in range(2):
                shift(i, 1, 2)
            shift(0, 2, 1)
            for i in range(2):
                shift(i, 1, 3)
            shift(0, 2, 2)
            shift(0, 2, 3)

            # --- steady: step s = dense(s) | sa(s-1) | sb(s-2) | y(s-3);
            # every shift MM depends only on previous-step DVE output.
            # The last three steps each pull one stage-quad forward
            # (sa same-step after the h copies, which is scalar- not
            # DVE-coupled) so the end-of-pipeline backlog halves. ---
            for s in range(4, NT - 3):
                if s + 2 < NT:
                    dma_xts(s + 2)
                for n in range(NCH):
                    dense_chunk(s, n)
                    shift(s - 1, 0, n)
                    shift(s - 2, 1, n)
                    shift(s - 3, 2, n)

            s = NT - 3      # + sa(s) same-step
            dma_xts(s + 2)
            for n in range(NCH):
                dense_chunk(s, n)
                shift(s - 1, 0, n)
                shift(s - 2, 1, n)
                shift(s - 3, 2, n)
                if n >= 2:
                    shift(s, 0, n - 2)
            shift(s, 0, 2)
            shift(s, 0, 3)

            s = NT - 2      # sa(s) same-step + sb(s-1) extra
            for n in range(NCH):
                dense_chunk(s, n)
                shift(s - 2, 1, n)
                shift(s - 3, 2, n)
                shift(s - 1, 1, n)
                if n >= 2:
                    shift(s, 0, n - 2)
            shift(s, 0, 2)
            shift(s, 0, 3)

            s = NT - 1      # sa(s) same-step + y(s-2) extra
            for n in range(NCH):
                dense_chunk(s, n)
                shift(s - 1, 1, n)
                shift(s - 3, 2, n)
                shift(s - 2, 2, n)
                if n >= 2:
                    shift(s, 0, n - 2)
            shift(s, 0, 2)
            shift(s, 0, 3)

            # --- drain: sb(16) | y(15), then y(16) ---
            for n in range(NCH):
                shift(NT - 1, 1, n)
                shift(NT - 2, 2, n)
            for n in range(NCH):
                shift(NT - 1, 2, n)

    _split_matmul_waits(nc)
    return nc


def _get_nc():
    if "nc" not in _CACHE:
        _CACHE["nc"] = _build_nc()
    return _CACHE["nc"]


def _block_xt(xn):
    # [NT*128, DIN]: block i row p, col 128k+c = x[hlo_i + c, 128k + p]
    xT3 = np.ascontiguousarray(xn.T).reshape(KD, 128, L)  # [k, p, t]
    out = np.empty((NT, 128, DIN), np.float32)
    for i in range(NT):
        _, _, hlo, _ = _tile_geom(i)
        # [k, p, c] -> [p, k, c]
        out[i] = xT3[:, :, hlo : hlo + 128].transpose(1, 0, 2).reshape(128, DIN)
    return out.reshape(NT * 128, DIN)


def _make_in_maps(x, W, b):
    x = np.asarray(x, np.float32)
    W = np.ascontiguousarray(np.asarray(W, np.float32))
    cst = _build_consts()
    return [
        {
            "xtb": _block_xt(x[n]),
            "w": W,
            "cst": cst,
        }
        for n in range(NCORES)
    ]


def kernel(x, W_dense, b_dense):
    from concourse.bass_utils import run_bass_kernel_spmd

    b = np.asarray(b_dense, np.float32)
    nc = _get_nc()
    in_maps = _make_in_maps(x, W_dense, b)
    res = run_bass_kernel_spmd(nc, in_maps, list(range(NCORES))).results

    y = np.stack([np.asarray(res[n]["y"]).astype(np.float32) for n in range(NCORES)])

    if np.any(b):
        # device kernel computes bias-free y; fold bias in on the host
        bext = np.concatenate([b, b[:PAD]])
        win = np.lib.stride_tricks.sliding_window_view(bext, WC).sum(axis=1)  # (F,)
        bias_y = np.broadcast_to(win, (L, F)).copy()
        for t in range(PAD):  # edge rows: only taps d <= t contribute
            bias_y[t] = np.array(
                [bext[o : o + t + 1].sum() for o in range(F)], np.float32
            )
        y = y + bias_y[None]

    a = np.tanh(y)
    z = (y > 0).astype(np.float32)
    return y, a, z


# revision 27
# speedup vs baseline: 1.0267x; 1.0099x over previous
"""Trainium2 Bass kernel for nn_ConvShiftLayer, v3.

Per batch element n (1 per NeuronCore, 8 cores):
    h = x[n] @ W_dense                                 (2048, 2048)
    y[t, o] = sum_{d=0..7} h[t-d, (o+d) % 2048]        (h[<0] = 0)
    a = tanh(y),  z = (y > 0)

v3 changes vs v2:
  - conv factorized log2-style: with (A_d f)[t,o] = f[t-d, o+d],
        y = (A0+A1)(A0+A2)(A0+A4) h
    Each stage is ONE PE shift-matmul (T_d row shift, +d col offset on
    the rhs) plus ONE row-aligned DVE add folding the identity term:
    3 shift MMs/chunk instead of 5 -> 44 MMs/tile vs 52.
  - device emits ONLY y (bf16). a = tanh(y) and z = (y > 0) are
    computed on the host from bf16 y (bf16 rounding is monotonic and
    sign-preserving, so z matches the device-fp32 z exactly).
  - lag-3 software pipeline: step s runs dense(s) | sa(s-1) | sb(s-2)
    | y(s-3), one (shift-MM, add) trio interleaved after each dense
    chunk, so no shift MM ever waits on a same-step DVE add.
  - prologue interleaves tiles 0-2's dense per W column chunk so the
    PE stays busy during the 8 MB W load.
  - PSUM: dense per-chunk 2 banks + 6-bank shift pool = 8.
  - output DMA on the gpsimd queue (inputs on the sync queue).
"""

import sys

if "/opt/trn_rl_repo" not in sys.path:
    sys.path.insert(0, "/opt/trn_rl_repo")

import numpy as np

B, L, DIN, F = 8, 2048, 1024, 2048
WC = 8            # conv taps
PAD = WC - 1      # 7
TS = 128 - PAD    # 121 output rows per time tile
NT = (L + TS - 1) // TS   # 17 time tiles
NCH = 4           # channel chunks of 512
CW = 512
NCORES = 8
KD = DIN // 128   # 8 K-tiles

SHIFTS = (4, 2, 1)             # stage shift amounts (T_4, T_2, T_1)
HSW = F + 7       # h tile cols: 2048 + 7 wrap (chain reads up to +7)
SASW = F + 3      # sa tile cols: 2048 + 3 wrap (sb reads +2, y +1)
SBSW = F + 1      # sb tile cols: 2048 + 1 wrap (y reads +1)

_CACHE = {}


def _build_consts():
    # cst[128, 384]: T_4 at [0:128), T_2 at [128:256), T_1 at [256:384)
    # T_d as lhsT: out[m] = rhs[m-d] (zero rows m<d give the h[t<0]=0
    # edge behavior for tile 0).
    c = np.zeros((128, 384), np.float32)
    for j, d in enumerate(SHIFTS):
        for m in range(128):
            if m - d >= 0:
                c[m - d, j * 128 + m] = 1.0
    return c


def _split_matmul_waits(nc):
    """This walrus build accepts only one sync-wait command per instruction;
    hoist extra waits onto preceding same-engine no-ops (one wait each)."""
    import concourse.mybir as mybir

    for fn in nc.m.functions:
        for blk in fn.blocks:
            newl = []
            for inst in blk.instructions:
                si = getattr(inst, "sync_info", None)
                if (
                    si is not None
                    and len(si.on_wait) > 1
                    and not isinstance(inst, mybir.InstNoOp)
                    and getattr(inst, "engine", None) is not None
                ):
                    waits = list(si.on_wait)
                    for wi, w in enumerate(waits[:-1]):
                        pre = mybir.InstNoOp(
                            name=f"{inst.name}_wsplit{wi}",
                            sync_info=mybir.SyncInfo(on_wait=[w], on_update=[]),
                            bass_nofuse=True,
                            engine=inst.engine,
                        )
                        newl.append(pre)
                    si.on_wait = waits[-1:]
                newl.append(inst)
            blk.instructions = newl


def _tile_geom(i):
    # uniform tiles: last tile overlaps tile 15 (identical values re-written)
    # so every tile outputs My=121 rows — the narrow-dtype consumer/DMA
    # path miscomputes on the hardware for shorter tiles.
    t0 = min(TS * i, L - TS)
    My = TS
    hlo = 0 if i == 0 else t0 - PAD
    Mh = min(L, t0 + TS) - hlo
    return t0, My, hlo, Mh


def _build_nc():
    import concourse.bass as bass
    import concourse.mybir as mybir
    from concourse import tile

    f32 = mybir.dt.float32
    bf16 = mybir.dt.bfloat16
    mmdt = mybir.dt.float32r

    nc = bass.Bass("TRN2", target_bir_lowering=False, debug=False)

    # pre-blocked x windows: row block i holds [128 p, 8 k x 128 c] with
    # element (p, 128k+c) = x[hlo_i + c, 128k + p]
    xt_d = nc.declare_dram_parameter("xtb", [NT * 128, DIN], f32, isOutput=False)
    w_d = nc.declare_dram_parameter("w", [DIN, F], f32, isOutput=False)
    cst_d = nc.declare_dram_parameter("cst", [128, 384], f32, isOutput=False)
    y_d = nc.declare_dram_parameter("y", [L, F], bf16, isOutput=True)

    with tile.TileContext(nc) as tc:
        with (
            tc.tile_pool(name="wpool", bufs=1) as wpool,
            tc.tile_pool(name="cpool", bufs=1) as cpool,
            tc.tile_pool(name="xtp", bufs=6) as xtp,
            tc.tile_pool(name="hpool", bufs=4) as hpool,
            tc.tile_pool(name="saspool", bufs=3) as saspool,
            tc.tile_pool(name="sbspool", bufs=3) as sbspool,
            tc.tile_pool(name="ybpool", bufs=3) as ybpool,
            tc.tile_pool(name="hppool", bufs=2, space="PSUM") as hppool,
            tc.tile_pool(name="shpool", bufs=6, space="PSUM") as shpool,
        ):
            halfd = DIN // 2
            xts = [None] * NT

            def dma_xts(i):
                xts[i] = xtp.tile([128, DIN], mmdt, tag="xts", name=f"xts{i}")
                nc.sync.dma_start(
                    xts[i][:, 0:halfd],
                    xt_d[i * 128 : (i + 1) * 128, 0:halfd].bitcast(mmdt),
                )
                nc.sync.dma_start(
                    xts[i][:, halfd:DIN],
                    xt_d[i * 128 : (i + 1) * 128, halfd:DIN].bitcast(mmdt),
                )

            wt = []
            for k in range(KD):
                wt.append(wpool.tile([128, F], mmdt, tag=f"w{k}", name=f"w{k}"))

            def dma_wcol(n, k0=0):
                for k in range(k0, KD):
                    nc.sync.dma_start(
                        wt[k][:, n * CW : (n + 1) * CW],
                        w_d[k * 128 : (k + 1) * 128, n * CW : (n + 1) * CW].bitcast(
                            mmdt
                        ),
                    )

            # input DMA order = first-consumption order; xts0's first
            # k-slice goes alone so the very first LDWEIGHTS can start
            # after only 64 KB + 256 KB of input
            xts[0] = xtp.tile([128, DIN], mmdt, tag="xts", name="xts0")
            nc.sync.dma_start(
                xts[0][:, 0:128], xt_d[0:128, 0:128].bitcast(mmdt)
            )
            nc.sync.dma_start(
                wt[0][:, 0:CW], w_d[0:128, 0:CW].bitcast(mmdt)
            )
            nc.sync.dma_start(
                xts[0][:, 128:halfd], xt_d[0:128, 128:halfd].bitcast(mmdt)
            )
            nc.sync.dma_start(
                xts[0][:, halfd:DIN], xt_d[0:128, halfd:DIN].bitcast(mmdt)
            )
            dma_wcol(0, k0=1)
            dma_xts(1)
            cst = cpool.tile([128, 384], mmdt, tag="cst")
            nc.sync.dma_start(cst[:], cst_d[:].bitcast(mmdt))
            dma_xts(2)
            dma_xts(3)
            dma_wcol(1)
            dma_wcol(2)
            dma_wcol(3)
            dma_xts(4)
            dma_xts(5)

            hs = [None] * NT
            sas = [None] * NT
            sbs = [None] * NT
            ybf = [None] * NT

            def lhsT(stage, Mh):
                # stage's shift matrix T_{SHIFTS[stage]} as lhsT [Mh, Mh]
                return cst[0:Mh, stage * 128 : stage * 128 + Mh]

            def dense_chunk(i, n):
                # dense(i, n): 8 accumulating k-MMs into one PSUM bank,
                # then scalar-drain to the h SBUF tile (+ wrap after n=3).
                _, _, _, Mh = _tile_geom(i)
                if hs[i] is None:
                    hs[i] = hpool.tile([128, HSW], mmdt, tag="hs", name=f"hs{i}")
                hp = hppool.tile([128, CW], f32, tag="hp")
                for k in range(KD):
                    nc.tensor.matmul(
                        hp[0:Mh, :],
                        xts[i][:, k * 128 : k * 128 + Mh],
                        wt[k][:, n * CW : (n + 1) * CW],
                        start=(k == 0),
                        stop=(k == KD - 1),
                    )
                nc.scalar.copy(hs[i][0:Mh, n * CW : (n + 1) * CW], hp[0:Mh, :])
                if n == NCH - 1:
                    nc.scalar.copy(hs[i][0:Mh, F:HSW], hs[i][0:Mh, 0 : HSW - F])

            def shift(i, stage, n):
                # stage 0: sa = h + A4 h; 1: sb = sa + A2 sa; 2: y = sb + A1 sb
                # one PE shift-MM into PSUM + one row-aligned DVE add.
                t0, My, hlo, Mh = _tile_geom(i)
                d = SHIFTS[stage]
                src = (hs, sas, sbs)[stage][i]
                sp = shpool.tile([128, CW], f32, tag="sp")
                nc.tensor.matmul(
                    sp[0:Mh, :],
                    lhsT(stage, Mh),
                    src[0:Mh, n * CW + d : n * CW + d + CW],
                    start=True,
                    stop=True,
                )
                if stage < 2:
                    dstl, wid, pool, tg = (
                        (sas, SASW, saspool, "sas")
                        if stage == 0
                        else (sbs, SBSW, sbspool, "sbs")
                    )
                    if dstl[i] is None:
                        dstl[i] = pool.tile(
                            [128, wid], mmdt, tag=tg, name=f"{tg}{i}"
                        )
                    dst = dstl[i]
                    nc.vector.tensor_tensor(
                        dst[0:Mh, n * CW : (n + 1) * CW],
                        src[0:Mh, n * CW : (n + 1) * CW].bitcast(f32),
                        sp[0:Mh, :],
                        mybir.AluOpType.add,
                    )
                    if n == NCH - 1:
                        nc.scalar.copy(dst[0:Mh, F:wid], dst[0:Mh, 0 : wid - F])
                else:
                    if ybf[i] is None:
                        ybf[i] = ybpool.tile([128, F], bf16, tag="ybf", name=f"yb{i}")
                    yb = ybf[i]
                    nc.vector.tensor_tensor(
                        yb[0:Mh, n * CW : (n + 1) * CW],
                        src[0:Mh, n * CW : (n + 1) * CW].bitcast(f32),
                        sp[0:Mh, :],
                        mybir.AluOpType.add,
                    )
                    # ship each half as soon as its adds are done so the
                    # last tile's DMA tail is short
                    if n == 1 or n == NCH - 1:
                        mlo = Mh - TS  # 0 for tile 0, 7 otherwise
                        cl = slice((n - 1) * CW, (n + 1) * CW)
                        nc.gpsimd.dma_start(
                            y_d[t0 : t0 + TS, cl], yb[mlo : mlo + TS, cl]
                        )

            # --- prologue: tiles 0-3 dense, k-major across tiles so each
            # arriving 256 KB W piece feeds 4 MMs (the W-load wire rate
            # is ~1 piece per 0.73 us vs 0.92 us of MMs); shifts
            # sa(0..2), sb(0..1), y(0) trail one column behind ---
            def dense_col_kmajor(tiles, n):
                hps = {}
                for i in tiles:
                    if hs[i] is None:
                        hs[i] = hpool.tile(
                            [128, HSW], mmdt, tag="hs", name=f"hs{i}"
                        )
                    hps[i] = hppool.tile(
                        [128, CW], f32, tag="hp", name=f"hp_p{i}_{n}"
                    )
                for k in range(KD):
                    for i in tiles:
                        _, _, _, Mh = _tile_geom(i)
                        nc.tensor.matmul(
                            hps[i][0:Mh, :],
                            xts[i][:, k * 128 : k * 128 + Mh],
                            wt[k][:, n * CW : (n + 1) * CW],
                            start=(k == 0),
                            stop=(k == KD - 1),
                        )
                for i in tiles:
                    _, _, _, Mh = _tile_geom(i)
                    nc.scalar.copy(
                        hs[i][0:Mh, n * CW : (n + 1) * CW], hps[i][0:Mh, :]
                    )
                    if n == NCH - 1:
                        nc.scalar.copy(
                            hs[i][0:Mh, F:HSW], hs[i][0:Mh, 0 : HSW - F]
                        )

            for n in range(NCH):
                for i in range(4):
                    dense_chunk(i, n)
                if n >= 1:
                    for i in range(3):
                        shift(i, 0, n - 1)
                if n >= 2:
                    for i in range(2):
                        shift(i, 1, n - 2)
                if n >= 3:
                    shift(0, 2, n - 3)
            for i in range(3):
                shift(i, 0, 3)
            for i in range(2):
                shift(i, 1, 2)
            shift(0, 2, 1)
            for i in range(2):
                shift(i, 1, 3)
            shift(0, 2, 2)
            shift(0, 2, 3)

            # --- steady: step s = dense(s) | sa(s-1) | sb(s-2) | y(s-3);
            # every shift MM depends only on previous-step DVE output.
            # The last three steps each pull one stage-quad forward
            # (sa same-step after the h copies, which is scalar- not
            # DVE-coupled) so the end-of-pipeline backlog halves. ---
            for s in range(4, NT - 3):
                if s + 2 < NT:
                    dma_xts(s + 2)
                for n in range(NCH):
                    dense_chunk(s, n)
                    shift(s - 1, 0, n)
                    shift(s - 2, 1, n)
                    shift(s - 3, 2, n)

            s = NT - 3      # + sa(s) same-step
            dma_xts(s + 2)
            for n in range(NCH):
                dense_chunk(s, n)
                shift(s - 1, 0, n)
                shift(s - 2, 1, n)
                shift(s - 3, 2, n)
                if n >= 1:
                    shift(s, 0, n - 1)
            shift(s, 0, 3)

            s = NT - 2      # sa(s) same-step + sb(s-1) extra
            for n in range(NCH):
                dense_chunk(s, n)
                shift(s - 2, 1, n)
                shift(s - 3, 2, n)
                shift(s - 1, 1, n)
                if n >= 1:
                    shift(s, 0, n - 1)
            shift(s, 0, 3)

            s = NT - 1      # sa(s) same-step + y(s-2) extra
            for n in range(NCH):
                dense_chunk(s, n)
                shift(s - 1, 1, n)
                shift(s - 3, 2, n)
                shift(s - 2, 2, n)
                if n >= 1:
                    shift(s, 0, n - 1)
            shift(s, 0, 3)

            # --- drain: sb(16) | y(15), then y(16) ---
            for n in range(NCH):
                shift(NT - 1, 1, n)
                shift(NT - 2, 2, n)
            for n in range(NCH):
                shift(NT - 1, 2, n)

    _split_matmul_waits(nc)
    return nc


def _get_nc():
    if "nc" not in _CACHE:
        _CACHE["nc"] = _build_nc()
    return _CACHE["nc"]


def _block_xt(xn):
    # [NT*128, DIN]: block i row p, col 128k+c = x[hlo_i + c, 128k + p]
    xT3 = np.ascontiguousarray(xn.T).reshape(KD, 128, L)  # [k, p, t]
    out = np.empty((NT, 128, DIN), np.float32)
    for i in range(NT):
        _, _, hlo, _ = _tile_geom(i)
        # [k, p, c] -> [p, k, c]
        out[i] = xT3[:, :, hlo : hlo + 128].transpose(1, 0, 2).reshape(128, DIN)
    return out.reshape(NT * 128, DIN)


def _make_in_maps(x, W, b):
    x = np.asarray(x, np.float32)
    W = np.ascontiguousarray(np.asarray(W, np.float32))
    cst = _build_consts()
    return [
        {
            "xtb": _block_xt(x[n]),
            "w": W,
            "cst": cst,
        }
        for n in range(NCORES)
    ]


def kernel(x, W_dense, b_dense):
    from concourse.bass_utils import run_bass_kernel_spmd

    b = np.asarray(b_dense, np.float32)
    nc = _get_nc()
    in_maps = _make_in_maps(x, W_dense, b)
    res = run_bass_kernel_spmd(nc, in_maps, list(range(NCORES))).results

    y = np.stack([np.asarray(res[n]["y"]).astype(np.float32) for n in range(NCORES)])

    if np.any(b):
        # device kernel computes bias-free y; fold bias in on the host
        bext = np.concatenate([b, b[:PAD]])
        win = np.lib.stride_tricks.sliding_window_view(bext, WC).sum(axis=1)  # (F,)
        bias_y = np.broadcast_to(win, (L, F)).copy()
        for t in range(PAD):  # edge rows: only taps d <= t contribute
            bias_y[t] = np.array(
                [bext[o : o + t + 1].sum() for o in range(F)], np.float32
            )
        y = y + bias_y[None]

    a = np.tanh(y)
    z = (y > 0).astype(np.float32)
    return y, a, z
